# revision 1
# baseline (speedup 1.0000x reference)
"""Channel-attention (CAM) Trainium2 Bass kernel.

Reference computation (per batch n):
    v = x[n].reshape(C, S)                 # C=512, S=H*W=4096
    energy = v @ v.T                       # (C, C)
    att = softmax(max_row(energy) - energy, axis=-1)
        = exp(min_row(energy) - energy) / Z
    out[n] = gamma * (att @ v) + x[n]

Sharding: data-parallel over N=8 batches across 8 NeuronCores; each core
computes one full C x C attention locally (no collectives).

Per-core plan (energy matmul in bf16 -- its +-200-range entries feed an
exp; the attention-weights matmul in fp8-e4m3 DoubleRow; f32 PSUM
accumulate everywhere). The timeline is DMA-bound (16.8 MiB at 360 GB/s
= 46.6 us); everything is organized to keep the serialized DMA engine
busy: loads back-to-back, then the first output group is produced as few
us after the last load as possible, then stores back-to-back.

  1. DMA x[n] (f32) -> SBUF in column stripes (7x512, 384, 128 -- the
     tail stripe is narrow so the last chunk's cast/transpose is short);
     high-priority DVE-cast to bf16 (vb) and GpSimd-cast to fp8 (vb8) as
     stripes land
  2. vT (bf16): ALL s-chunks are transposed on the PE as their stripes
     land; no xbar DMA transposes (they would serialize after the loads
     and add ~5us to the critical path). The last TAILF32 chunks are
     transposed from xf directly in f32 (2 cycles/row; the PE has slack
     there) so the tail needs no bf16 cast step at all -- the PSUM->SBUF
     copy does the f32->bf16 cast. Copies go to ACT for the first
     ACT_VBT chunks and every other tail chunk (keeping both ACT and DVE
     partially free near the tail), DVE otherwise (cast 327 + copy 392 <
     728ns/chunk DMA rate)
  3. einsum1 energy = sum_k vT_k.T @ vT_k (bf16), UPPER BLOCK-TRIANGLE
     ONLY (62.5% of the work), accumulated into 4 PSUM banks; chunks
     below KPE accumulate inline during the loads, emitted LAG chunks
     behind the transposes so the PE never waits on a copy it just
     triggered; the tail chunks' matmuls are emitted ci-outer (row 0
     first) after the load loop, so row 0's group closes right after the
     last vbT lands and its softmax starts immediately. The 6 lower
     blocks are reconstructed with f32 PE transposes of finished upper
     blocks
  4. per row-block pipeline: DVE row-min -> ACT exp (accum_out = row
     sums Z) -> dsc = diag(gamma/Z) built by row-scaling the identity
     (DVE reciprocal + one fused tensor_scalar double-multiply) -> 4 PE
     matmuls P_blk^T @ dsc (a regular matmul, NOT transpose mode: the
     transpose datapath requires a permutation rhs and ignores its
     values), folding the gamma/Z scale into P^T for free -> ONE strided
     ACT copy PSUM->SBUF casts all 4 blocks to fp8 -> einsum2
     out2 = PT.T @ vb8 as fp8 DoubleRow pair-matmuls (output is already
     scaled) -> add x in place into xf (DVE reads PSUM directly for most
     groups; POOL_SJ groups per row bounce through an ACT copy so the
     GpSimd engine can carry them) -> DMA out from xf; trailing einsum2
     groups are deferred into the next row-block's softmax window so the
     store DMA never starves
Cost-model (TimelineSim): 61377 ns/core (baseline 64056) vs ~47 us DMA
roofline; the gap is the load->first-store latency (tail-chunk
cast/transpose/copy pipeline + row-0 softmax chain + first store's
HWDGE/DGE issue pipeline, each link paying semaphore propagation).
"""

import os
from contextlib import ExitStack

import numpy as np

import concourse.bass as bass
import concourse.tile as tile
from concourse import bacc, mybir
from concourse.bass_utils import run_bass_kernel_spmd
from concourse.masks import make_identity

N_CORES = 8
C = 512
S = 4096
P = 128
CI = C // P  # 4 c-chunks
KD = S // P  # 32 s-chunks of 128
SJW = 512
SJ = S // SJW  # 8 s-chunks of 512

# Narrow LAST stripe only: the final chunk's cast/transpose/matmul tail is
# short. Narrow stripes anywhere else lose DMA time to the per-copy issue
# overhead (650ns SP issue / 625ns HWDGE vs 364ns transfer), and 9 stripes
# = 36 dma_starts is the most the SP sequencer can issue inside the load
# window.
STRIPES = [512] * 7 + [384, 128]
# einsum1 chunks accumulated inline during the load phase. Kept BELOW the
# DMA-rate-matched value (~29): the PE runs mid-p-state (533ns/chunk of
# einsum1 + 212ns of transposes > 728ns DMA), so a rate-matched phase 1
# would finish ~6us after the loads; with 22 the PE idles briefly mid-load
# and every row's 8-chunk phase-2 tail closes right as its last vbT lands.
KPE = 22
LAG = 2         # chunks the inline einsum1 trails the transposes by
DEFER = 2       # einsum2 groups deferred into the next row's softmax window
POOL_SJ = (3, 5, 6, 7)  # groups routed ACT-copy + GpSimd-add (no PSUM port on GpSimd)
ACT_VBT = 12     # leading chunks whose PSUM->SBUF copy goes on ACT, not DVE
TAILF32 = 6     # trailing chunks transposed from xf in f32 (no bf16 cast)

F32 = mybir.dt.float32
BF16 = mybir.dt.bfloat16
FP8 = mybir.dt.float8e4


def _body(ctx: ExitStack, tc: tile.TileContext, out: bass.AP, x: bass.AP,
          gamma: bass.AP):
    nc = tc.nc

    persist = ctx.enter_context(tc.tile_pool(name="persist", bufs=1))
    xf = persist.tile([P, CI, S], F32, name="xf")
    vb = persist.tile([P, CI, S], BF16, name="vb")
    vb8 = persist.tile([P, CI, S], FP8, name="vb8")
    vbT = persist.tile([P, KD, C], BF16, name="vbT")
    p_sb = persist.tile([P, CI, C], BF16, name="p_sb")
    pt_sb = persist.tile([P, CI, C], FP8, name="pt_sb")
    ident = persist.tile([P, P], BF16, name="ident")
    identf = persist.tile([P, P], F32, name="identf")
    gamma_sb = persist.tile([P, 1], F32, name="gamma_sb")
    mn = persist.tile([P, CI], F32, name="mn")
    zsum = persist.tile([P, CI], F32, name="zsum")
    msc = persist.tile([P, CI], F32, name="msc")

    make_identity(nc, ident)
    make_identity(nc, identf)

    x3 = x.rearrange("(ci p) s -> p ci s", p=P)
    out3 = out.rearrange("(ci p) s -> p ci s", p=P)

    opool = ctx.enter_context(tc.tile_pool(name="opool", bufs=4, space="PSUM"))
    epool = ctx.enter_context(tc.tile_pool(name="epool", bufs=4, space="PSUM"))
    e_ps = [epool.tile([P, C], F32, name=f"e{ci}", tag="et") for ci in range(CI)]
    otmp_pool = ctx.enter_context(tc.tile_pool(name="otmp", bufs=6))
    dsc_pool = ctx.enter_context(tc.tile_pool(name="dscp", bufs=2))

    # ---- load + cast + PE-transpose + einsum1, striped ----
    # Each chunk's pipeline: DMA stripe -> DVE bf16 cast -> 4 PE transposes
    # -> PSUM->SBUF copy -> 4 upper-triangle einsum1 matmuls.
    col = 0
    for si, w in enumerate(STRIPES):
        sl = slice(col, col + w)
        for ci in range(CI):
            nc.sync.dma_start(out=xf[:, ci, sl], in_=x3[:, ci, sl])
            # high priority: a cast must run the moment its DMA lands --
            # casts emitted late would otherwise queue behind older vbT
            # copies in DVE's ready heap. The tail chunks (k >= KD-TAILF32)
            # need no bf16 cast at all: their transposes read xf directly
            # in f32 and the PSUM->SBUF copy does the cast.
            if col < (KD - TAILF32) * P:
                with tc.high_priority():
                    nc.vector.tensor_copy(out=vb[:, ci, sl],
                                          in_=xf[:, ci, sl])
            # fp8 copy of v for einsum2's DoubleRow matmuls; GpSimd is idle
            # during the load phase
            nc.gpsimd.tensor_copy(out=vb8[:, ci, sl], in_=xf[:, ci, sl])
        if si == 0:
            # needed only at softmax time; issued behind the first stripe
            # so it cannot delay the x loads
            nc.sync.dma_start(out=gamma_sb[:, :],
                              in_=gamma.to_broadcast((P, 1)))
        for k in range(col // P, (col + w) // P):
            tailf = k >= KD - TAILF32
            # tail chunks transpose from xf in f32 (2 cycles/row on the PE,
            # which has slack there) so the chain is DMA -> transpose with
            # no cast step; the copy below casts f32 PSUM -> bf16 SBUF
            tp_ps = opool.tile([P, C], F32 if tailf else BF16,
                               name="tp_ps", tag="op")
            for ci in range(CI):
                nc.tensor.transpose(
                    out=tp_ps[:, ci * P:(ci + 1) * P],
                    in_=(xf if tailf else vb)[:, ci, k * P:(k + 1) * P],
                    identity=(identf if tailf else ident)[:, :],
                )
            # ACT takes the first few copies (it is idle early) and every
            # other TAIL copy (it is idle late); DVE carries the steady
            # middle (cast 327 + copy 392 = 719ns/chunk < 728ns DMA rate).
            # A single-engine tail would serialize ~4.7us of cast+copy work
            # behind the last loads.
            if k < ACT_VBT or (k >= KPE and k % 2 == 0):
                nc.scalar.copy(out=vbT[:, k, :], in_=tp_ps[:, :])
            else:
                nc.vector.tensor_copy(out=vbT[:, k, :], in_=tp_ps[:, :])
            # energy is symmetric: row-block ci keeps columns >= ci*P
            # (upper block-triangle, 62.5% of the matmul work). Only chunks
            # below KPE accumulate here (phase 1), emitted LAG chunks behind
            # the transposes so the PE (in-order queue) never waits on the
            # PSUM->SBUF vbT copy it just triggered; the tail chunks' matmuls
            # run ci-outer in phase 2, so row 0's accumulation group closes
            # (and its softmax starts) right after the last load instead of
            # behind every other row's matmuls.
            km = k - LAG
            if 0 <= km < KPE:
                for ci in range(CI):
                    nc.tensor.matmul(
                        e_ps[ci][:, ci * P:],
                        lhsT=vbT[:, km, ci * P:(ci + 1) * P],
                        rhs=vbT[:, km, ci * P:],
                        start=(km == 0),
                        stop=False,
                    )
        col += w

    def einsum2_group(ci, sj, lo, hi):
        # fp8 DoubleRow: each matmul contracts a PAIR of 128-d chunks.
        # PT is pre-scaled by gamma/Z (folded into the transpose), so the
        # epilogue is a single add of x.
        w = hi - lo
        o_ps = opool.tile([P, w], F32, name="o_ps", tag="op")
        for h in range(CI // 2):
            nc.tensor.matmul(
                o_ps[:, :],
                lhsT=pt_sb[:, 2 * h:2 * h + 2, ci * P:(ci + 1) * P],
                rhs=vb8[:, 2 * h:2 * h + 2, lo:hi],
                start=(h == 0),
                stop=(h == CI // 2 - 1),
                perf_mode=mybir.MatmulPerfMode.DoubleRow,
            )
        hs = slice(lo, hi)
        if sj in POOL_SJ:
            # GpSimd has no PSUM port: bounce through SBUF on ACT
            o_tmp = otmp_pool.tile([P, w], F32, name="o_tmp", tag="ot")
            nc.scalar.copy(out=o_tmp[:, :], in_=o_ps[:, :])
            nc.gpsimd.tensor_add(out=xf[:, ci, hs], in0=o_tmp[:, :],
                                 in1=xf[:, ci, hs])
        else:
            nc.vector.tensor_add(out=xf[:, ci, hs], in0=o_ps[:, :],
                                 in1=xf[:, ci, hs])
        nc.sync.dma_start(out=out3[:, ci, hs], in_=xf[:, ci, hs])

    def group_ranges(ci):
        return [(sj * SJW, (sj + 1) * SJW) for sj in range(SJ)]

    # phase 2: close every row's accumulation group, then reconstruct the
    # lower triangle, BEFORE any softmax -- all 4 rows are ready ~2us after
    # the last load, so the per-row softmax/einsum2 pipeline below is never
    # blocked on the PE.
    for ci in range(CI):
        for k in range(KPE, KD):
            nc.tensor.matmul(
                e_ps[ci][:, ci * P:],
                lhsT=vbT[:, k, ci * P:(ci + 1) * P],
                rhs=vbT[:, k, ci * P:],
                start=False,
                stop=(k == KD - 1),
            )
    deferred = []
    for ci in range(CI):
        # reconstruct this row's lower blocks: e[ci, cj<ci] = e[cj, ci].T
        # (after both rows' accumulation groups close -- writing a transpose
        # into a bank with an open matmul group corrupts it). Row 0 has none,
        # so its softmax chain starts unencumbered.
        for cj in range(ci):
            tr_sb = otmp_pool.tile([P, P], F32, name="tr_sb", tag="tr", bufs=3)
            nc.scalar.copy(out=tr_sb[:, :],
                           in_=e_ps[cj][:, ci * P:(ci + 1) * P])
            nc.tensor.matmul(
                e_ps[ci][:, cj * P:(cj + 1) * P],
                lhsT=tr_sb[:, :],
                rhs=identf[:, :],
                is_transpose=True,
                skip_group_check=True,
            )
        for args in deferred:  # keeps the store DMA fed during the softmax
            einsum2_group(*args)
        deferred = []
        # softmax (reversed): P = exp(min_row(e) - e), Z = row sums
        nc.vector.tensor_reduce(
            out=mn[:, ci:ci + 1], in_=e_ps[ci][:, :],
            axis=mybir.AxisListType.X, op=mybir.AluOpType.min,
        )
        nc.scalar.activation(
            out=p_sb[:, ci, :], in_=e_ps[ci][:, :],
            func=mybir.ActivationFunctionType.Exp,
            bias=mn[:, ci:ci + 1], scale=-1.0,
            accum_out=zsum[:, ci:ci + 1],
        )
        nc.vector.reciprocal(out=msc[:, ci:ci + 1], in_=zsum[:, ci:ci + 1])
        # dsc = diag(gamma/Z): row-scale the identity, fusing the two
        # multiplies ((ident * 1/Z) * gamma). Using dsc instead of the
        # identity in the PT transposes scales P^T's columns (the c axis)
        # by gamma/Z, so einsum2's output needs no per-group scale pass.
        dsc = dsc_pool.tile([P, P], BF16, name="dsc", tag="dsc")
        nc.vector.tensor_scalar(
            out=dsc[:, :], in0=ident[:, :],
            scalar1=msc[:, ci:ci + 1], scalar2=gamma_sb[:, :],
            op0=mybir.AluOpType.mult, op1=mybir.AluOpType.mult,
        )
        # PT block = P_block^T @ diag(msc) as a REGULAR matmul (the PE
        # transpose datapath requires a permutation rhs and ignores values,
        # so the scale must go through the normal matmul path; f32 out)
        pt_ps = opool.tile([P, CI, P], F32, name="pt_ps", tag="op")
        for dj in range(CI):
            nc.tensor.matmul(
                pt_ps[:, dj, :],
                lhsT=p_sb[:, ci, dj * P:(dj + 1) * P],
                rhs=dsc[:, :],
            )
        # one strided copy casts all 4 blocks to fp8 for the DoubleRow lhsT
        nc.scalar.copy(out=pt_sb[:, :, ci * P:(ci + 1) * P],
                       in_=pt_ps[:, :, :])

        ranges = group_ranges(ci)
        keep = len(ranges) if ci == CI - 1 else len(ranges) - DEFER
        for sj in range(keep):
            einsum2_group(ci, sj, *ranges[sj])
        deferred = [(ci, sj) + ranges[sj] for sj in range(keep, len(ranges))]


def build():
    nc = bacc.Bacc("TRN2", target_bir_lowering=False, debug=False,
                   num_devices=N_CORES)
    x = nc.dram_tensor("x", [C, S], F32, kind="ExternalInput")
    gamma = nc.dram_tensor("gamma", [1], F32, kind="ExternalInput")
    out = nc.dram_tensor("out", [C, S], F32, kind="ExternalOutput")
    with tile.TileContext(nc) as tc:
        with ExitStack() as ctx:
            _body(ctx, tc, out.ap(), x.ap(), gamma.ap())
    nc.compile()
    return nc


_NC_CACHE = {}
LAST_RESULTS = None


def kernel(x: np.ndarray, gamma: np.ndarray) -> np.ndarray:
    global LAST_RESULTS
    x = np.ascontiguousarray(np.asarray(x, dtype=np.float32))
    gamma = np.ascontiguousarray(np.asarray(gamma, dtype=np.float32))
    n, c, h, w = x.shape
    assert (n, c, h * w) == (N_CORES, C, S), f"unexpected shape {x.shape}"

    # NTFF tracing is unavailable through this execution path; make sure an
    # inherited BASS_TRACE=1 cannot divert run_bass_kernel_spmd into it.
    os.environ["BASS_NEVER_TRACE"] = "1"

    if "nc" not in _NC_CACHE:
        _NC_CACHE["nc"] = build()
    nc = _NC_CACHE["nc"]

    in_maps = [
        {"x": x[i].reshape(C, S), "gamma": gamma} for i in range(N_CORES)
    ]
    res = run_bass_kernel_spmd(nc, in_maps, core_ids=list(range(N_CORES)))
    LAST_RESULTS = res
    out = np.stack([res.results[i]["out"] for i in range(N_CORES)], axis=0)
    return out.reshape(n, c, h, w).astype(np.float32, copy=False)


if __name__ == "__main__":
    xs = np.random.randn(N_CORES, C, 64, 64).astype(np.float32)
    g = np.zeros((1,), np.float32)
    o = kernel(xs, g)
    print("ok", o.shape, np.abs(o - xs).max())



# revision 6
# speedup vs baseline: 1.0072x; 1.0072x over previous
"""Channel-attention (CAM) Trainium2 Bass kernel.

Reference computation (per batch n):
    v = x[n].reshape(C, S)                 # C=512, S=H*W=4096
    energy = v @ v.T                       # (C, C)
    att = softmax(max_row(energy) - energy, axis=-1)
        = exp(min_row(energy) - energy) / Z
    out[n] = gamma * (att @ v) + x[n]

Sharding: data-parallel over N=8 batches across 8 NeuronCores; each core
computes one full C x C attention locally (no collectives).

v2 restructure (from the 61377ns baseline): the output is stored as
BF16 (rel-err ~2e-3 vs the 2e-2 gate), halving store DMA from 23.3us to
11.65us. That makes the drain engine-bound instead of DMA-bound: every
einsum2 output group must cross PSUM->SBUF through DVE (~1352ns/1024) or
ACT (~1257ns/1024) -- gpsimd has no PSUM port and DMA cannot read PSUM.
So the drain is organized around feeding those two engines:

  - einsum2 emits 1024-wide PSUM groups (two banks per tile; the wide
    read amortizes the fixed PSUM-access cost ~17-25%/group).
  - Route A groups: one DVE tensor_add (PSUM + xf f32 -> vb bf16).
  - Route B groups: the +x fold happens on the PE (a bf16 identity
    matmul accumulating x into the same PSUM group), then one ACT copy
    PSUM -> vb bf16. This costs the idle PE ~245ns/group and relieves
    DVE entirely.
  - gpsimd carries everything scalar: the fp8 casts of v (load phase),
    and dsc = (gamma*I)/Z built with a single divide tensor_scalar (no
    DVE reciprocal, no extra mult).
  - PSUM->vbT copies in the load phase run on PAIRS of chunks (one
    1024-wide copy per two transposed chunks), halving per-op overhead
    and per-pair sem traffic; pairs alternate DVE/ACT.
  - PSUM layout: 4 banks energy rows (epool) + 2 rotating 4KB slots
    (opool, tag-shared by the transpose staging pairs, the 1024-wide
    einsum2 groups and the PT staging tile -- they are temporally
    disjoint).

The +x term reaches the output through bf16 exactly once (either the
DVE add's cast or vb's load-phase cast), so gamma=0 output error is pure
bf16 rounding (~2e-3).  Stores are 1024-wide (728ns transfer > 632ns
HWDGE issue, so the issue pipeline keeps up), all on the SP queue which
is idle after the 36 load DMAs.
"""

import os
from contextlib import ExitStack

import numpy as np

import concourse.bass as bass
import concourse.tile as tile
from concourse import bacc, mybir
from concourse.bass_utils import run_bass_kernel_spmd
from concourse.masks import make_identity

N_CORES = 8
C = 512
S = 4096
P = 128
CI = C // P  # 4 c-chunks
KD = S // P  # 32 s-chunks of 128
OW = 1024    # einsum2 output group width (2 PSUM banks)
OG = S // OW  # 4 groups per row

STRIPES = [512] * 7 + [384, 128]
KPE = 22        # chunks whose einsum1 runs all 4 rows inline during loads
TAILF32 = 6     # trailing chunks transposed from xf in f32 (no bf16 cast)
# einsum2 route per group: 'A' = DVE add (PSUM+xf->vb), 'B' = PE identity
# add + ACT copy. g3 must be 'A' (its columns overlap the TAILF32 region,
# which never gets a bf16 cast into vb).
ROUTES = ("B", "A", "B", "A")
# engine for each vbT pair copy: 'D' (DVE) / 'A' (ACT); pairs 13..15 are
# the f32 tail pairs.
PAIR_ENG = list("DADADADADADAAAAA")
# engine for each of the 36 (stripe, ci) fp8 casts: mostly gpsimd; a few
# spill to DVE/ACT where they have load-phase slack.
VB8_ENG = ["G"] * 36

F32 = mybir.dt.float32
BF16 = mybir.dt.bfloat16
FP8 = mybir.dt.float8e4


def _body(ctx: ExitStack, tc: tile.TileContext, out: bass.AP, x: bass.AP,
          gamma: bass.AP):
    nc = tc.nc

    persist = ctx.enter_context(tc.tile_pool(name="persist", bufs=1))
    xf = persist.tile([P, CI, S], F32, name="xf")
    vb = persist.tile([P, CI, S], BF16, name="vb")   # bf16 x, then output
    vb8 = persist.tile([P, CI, S], FP8, name="vb8")
    vbT = persist.tile([P, KD, C], BF16, name="vbT")
    p_sb = persist.tile([P, CI, C], BF16, name="p_sb")
    pt_sb = persist.tile([P, CI, C], FP8, name="pt_sb")
    ident = persist.tile([P, P], BF16, name="ident")
    identf = persist.tile([P, P], F32, name="identf")
    identg = persist.tile([P, P], BF16, name="identg")
    gamma_sb = persist.tile([P, 1], F32, name="gamma_sb")
    mn = persist.tile([P, CI], F32, name="mn")
    zsum = persist.tile([P, CI], F32, name="zsum")
    msc = persist.tile([P, CI], F32, name="msc")

    make_identity(nc, ident)
    make_identity(nc, identf)

    x3 = x.rearrange("(ci p) s -> p ci s", p=P)
    out3 = out.rearrange("(ci p) s -> p ci s", p=P)

    epool = ctx.enter_context(tc.tile_pool(name="epool", bufs=4, space="PSUM"))
    e_ps = [epool.tile([P, C], F32, name=f"e{ci}", tag="et") for ci in range(CI)]
    # 2 rotating 4KB slots shared (by tag) between transpose-staging pairs,
    # 1024-wide einsum2 groups and the PT staging tile.
    opool = ctx.enter_context(tc.tile_pool(name="opool", bufs=2, space="PSUM"))
    trpool = ctx.enter_context(tc.tile_pool(name="trp", bufs=3))
    dscpool = ctx.enter_context(tc.tile_pool(name="dscp", bufs=2))

    # ---- load + cast + PE-transpose + inline einsum1 ----
    def emit_e1(k):
        """einsum1 matmuls for chunk k: all 4 rows below KPE, row 0 only
        after (rows 1-3 of the tail run in phase 2 so row 0's group --
        and its softmax -- closes right after the last load)."""
        rows = range(CI) if k < KPE else (0,)
        for ci in rows:
            nc.tensor.matmul(
                e_ps[ci][:, ci * P:],
                lhsT=vbT[:, k, ci * P:(ci + 1) * P],
                rhs=vbT[:, k, ci * P:],
                start=(k == 0),
                stop=(k == KD - 1),
            )

    tp_cur = None
    col = 0
    ncast = 0
    for si, w in enumerate(STRIPES):
        sl = slice(col, col + w)
        for ci in range(CI):
            nc.sync.dma_start(out=xf[:, ci, sl], in_=x3[:, ci, sl])
            cast_hi = min(col + w, (KD - TAILF32) * P)
            if col < cast_hi:
                with tc.high_priority():
                    nc.vector.tensor_copy(out=vb[:, ci, col:cast_hi],
                                          in_=xf[:, ci, col:cast_hi])
            eng = VB8_ENG[si * CI + ci]
            e8 = {"G": nc.gpsimd, "D": nc.vector, "A": nc.scalar}[eng]
            if eng == "A":
                e8.copy(out=vb8[:, ci, sl], in_=xf[:, ci, sl])
            else:
                e8.tensor_copy(out=vb8[:, ci, sl], in_=xf[:, ci, sl])
        if si == 0:
            nc.sync.dma_start(out=gamma_sb[:, :],
                              in_=gamma.to_broadcast((P, 1)))
            # gamma*I, built once; dsc = identg * (1/Z) later
            nc.vector.tensor_scalar(
                out=identg[:, :], in0=ident[:, :], scalar1=gamma_sb[:, :],
                scalar2=None, op0=mybir.AluOpType.mult,
            )
        for k in range(col // P, (col + w) // P):
            tailf = k >= KD - TAILF32
            j, half = k // 2, k % 2
            if half == 0:
                tp_cur = opool.tile([P, 2, C], F32 if tailf else BF16,
                                    name="tp", tag="op")
            for ci in range(CI):
                nc.tensor.transpose(
                    out=tp_cur[:, half, ci * P:(ci + 1) * P],
                    in_=(xf if tailf else vb)[:, ci, k * P:(k + 1) * P],
                    identity=(identf if tailf else ident)[:, :],
                )
            if half == 1:
                # one 1024-wide PSUM->SBUF copy moves the whole pair
                eng = nc.vector if PAIR_ENG[j] == "D" else nc.scalar
                if PAIR_ENG[j] == "D":
                    eng.tensor_copy(out=vbT[:, 2 * j:2 * j + 2, :],
                                    in_=tp_cur[:, :, :])
                else:
                    eng.copy(out=vbT[:, 2 * j:2 * j + 2, :],
                             in_=tp_cur[:, :, :])
                # einsum1 for the PREVIOUS pair: its copy completed while
                # this pair's transposes ran, so the in-order PE never
                # stalls on a copy it just triggered.
                if j > 0:
                    emit_e1(2 * (j - 1))
                    emit_e1(2 * (j - 1) + 1)
        col += w
        ncast += 1
    emit_e1(KD - 2)
    emit_e1(KD - 1)

    # ---- phase 2 + per-row softmax / einsum2 / store ----
    def phase2(ci):
        for k in range(KPE, KD):
            nc.tensor.matmul(
                e_ps[ci][:, ci * P:],
                lhsT=vbT[:, k, ci * P:(ci + 1) * P],
                rhs=vbT[:, k, ci * P:],
                start=False,
                stop=(k == KD - 1),
            )

    def softmax_and_groups(ci):
        # reconstruct lower blocks from the (closed) upper rows
        for cj in range(ci):
            tr_sb = trpool.tile([P, P], F32, name="tr_sb", tag="tr")
            if (ci + cj) % 2 == 0:
                nc.scalar.copy(out=tr_sb[:, :],
                               in_=e_ps[cj][:, ci * P:(ci + 1) * P])
            else:
                nc.vector.tensor_copy(out=tr_sb[:, :],
                                      in_=e_ps[cj][:, ci * P:(ci + 1) * P])
            nc.tensor.matmul(
                e_ps[ci][:, cj * P:(cj + 1) * P],
                lhsT=tr_sb[:, :],
                rhs=identf[:, :],
                is_transpose=True,
                skip_group_check=True,
            )
        # softmax (reversed): P = exp(min_row(e) - e), Z = row sums
        nc.vector.tensor_reduce(
            out=mn[:, ci:ci + 1], in_=e_ps[ci][:, :],
            axis=mybir.AxisListType.X, op=mybir.AluOpType.min,
        )
        nc.scalar.activation(
            out=p_sb[:, ci, :], in_=e_ps[ci][:, :],
            func=mybir.ActivationFunctionType.Exp,
            bias=mn[:, ci:ci + 1], scale=-1.0,
            accum_out=zsum[:, ci:ci + 1],
        )
        # dsc = (gamma*I) * (1/Z)
        nc.vector.reciprocal(out=msc[:, ci:ci + 1], in_=zsum[:, ci:ci + 1])
        dsc = dscpool.tile([P, P], BF16, name="dsc", tag="dsc")
        nc.vector.tensor_scalar(
            out=dsc[:, :], in0=identg[:, :], scalar1=msc[:, ci:ci + 1],
            scalar2=None, op0=mybir.AluOpType.mult,
        )
        # PT block = P_block^T @ diag(gamma/Z) (regular matmul: the
        # transpose datapath ignores rhs values, so the scale must go
        # through the normal path)
        pt_ps = opool.tile([P, CI, P], F32, name="pt_ps", tag="op")
        for dj in range(CI):
            nc.tensor.matmul(
                pt_ps[:, dj, :],
                lhsT=p_sb[:, ci, dj * P:(dj + 1) * P],
                rhs=dsc[:, :],
            )
        if ci % 2 == 0:
            nc.scalar.copy(out=pt_sb[:, :, ci * P:(ci + 1) * P],
                           in_=pt_ps[:, :, :])
        else:
            nc.vector.tensor_copy(out=pt_sb[:, :, ci * P:(ci + 1) * P],
                                  in_=pt_ps[:, :, :])

        for g in range(OG):
            lo = g * OW
            route = ROUTES[g]
            o_ps = opool.tile([P, OW], F32, name="o_ps", tag="op")
            for hb in range(OW // 512):
                hlo = lo + hb * 512
                hsl = slice(hlo, hlo + 512)
                psl = slice(hb * 512, hb * 512 + 512)
                for h in range(CI // 2):
                    nc.tensor.matmul(
                        o_ps[:, psl],
                        lhsT=pt_sb[:, 2 * h:2 * h + 2, ci * P:(ci + 1) * P],
                        rhs=vb8[:, 2 * h:2 * h + 2, hsl],
                        start=(h == 0),
                        stop=(h == CI // 2 - 1 and route == "A"),
                        perf_mode=mybir.MatmulPerfMode.DoubleRow,
                    )
                if route == "B":
                    # fold +x on the PE: accumulate I.T @ bf16(x) into the
                    # same group, so the drain is a bare ACT copy
                    nc.tensor.matmul(
                        o_ps[:, psl],
                        lhsT=ident[:, :],
                        rhs=vb[:, ci, hsl],
                        start=False,
                        stop=True,
                        skip_group_check=True,
                    )
            gsl = slice(lo, lo + OW)
            if route == "A":
                nc.vector.tensor_add(out=vb[:, ci, gsl], in0=o_ps[:, :],
                                     in1=xf[:, ci, gsl])
            else:
                nc.scalar.copy(out=vb[:, ci, gsl], in_=o_ps[:, :])
            nc.sync.dma_start(out=out3[:, ci, gsl], in_=vb[:, ci, gsl])

    phase2(1)
    softmax_and_groups(0)
    phase2(2)
    softmax_and_groups(1)
    phase2(3)
    softmax_and_groups(2)
    softmax_and_groups(3)


def build():
    nc = bacc.Bacc("TRN2", target_bir_lowering=False, debug=False,
                   num_devices=N_CORES)
    x = nc.dram_tensor("x", [C, S], F32, kind="ExternalInput")
    gamma = nc.dram_tensor("gamma", [1], F32, kind="ExternalInput")
    out = nc.dram_tensor("out", [C, S], BF16, kind="ExternalOutput")
    with tile.TileContext(nc) as tc:
        with ExitStack() as ctx:
            _body(ctx, tc, out.ap(), x.ap(), gamma.ap())
    nc.compile()
    return nc


_NC_CACHE = {}
LAST_RESULTS = None


def kernel(x: np.ndarray, gamma: np.ndarray) -> np.ndarray:
    global LAST_RESULTS
    x = np.ascontiguousarray(np.asarray(x, dtype=np.float32))
    gamma = np.ascontiguousarray(np.asarray(gamma, dtype=np.float32))
    n, c, h, w = x.shape
    assert (n, c, h * w) == (N_CORES, C, S), f"unexpected shape {x.shape}"

    os.environ["BASS_NEVER_TRACE"] = "1"

    if "nc" not in _NC_CACHE:
        _NC_CACHE["nc"] = build()
    nc = _NC_CACHE["nc"]

    in_maps = [
        {"x": x[i].reshape(C, S), "gamma": gamma} for i in range(N_CORES)
    ]
    res = run_bass_kernel_spmd(nc, in_maps, core_ids=list(range(N_CORES)))
    LAST_RESULTS = res
    out = np.stack(
        [np.asarray(res.results[i]["out"]) for i in range(N_CORES)], axis=0
    ).astype(np.float32)
    return out.reshape(n, c, h, w)


if __name__ == "__main__":
    xs = np.random.randn(N_CORES, C, 64, 64).astype(np.float32)
    g = np.zeros((1,), np.float32)
    o = kernel(xs, g)
    print("ok", o.shape, np.abs(o - xs).max())
